# revision 12
# baseline (speedup 1.0000x reference)
"""ConformerDecoder Trainium2 Bass kernel.

Sharding: pure data-parallel over batch B=8 -> one sample per NeuronCore.

All activations live in "transposed" layout [feature-on-partitions, T-free]
so the entire matmul chain (ffn1 -> qkv -> attn -> out-proj -> conv pw1 ->
pw2 -> ffn2) runs with weights as natural lhsT operands and zero activation
transposes.  bf16 matmuls with fp32 PSUM accumulation.

Host-side one-time model formatting: cast weights to bf16, fold constant
scales (0.25 into ffn w2 = silu-half * residual-half; 0.5 into pw2 =
conv-silu half; 0.5 into dw = GLU-sigmoid half; 1/sqrt(DH) into Wq), build
band masks / identity / ones as inline const tensors.

LayerNorm (feature axis = partitions): per-token sums via ones[128,128]
matmuls on PE (output rows are 128-replicated broadcasts for free), rsqrt
via seeded Newton on DVE.  silu/sigmoid via Tanh so a single ACT table set
(exp_and_others: Exp+Tanh) serves the whole kernel -- no table switches.

Windowed attention (W=64): per (head, q-block of 128): one scores matmul
against a 3-chunk (384-wide) zero-padded K window, full-tile Exp on ACT,
multiplicative band-mask scalar_tensor_tensor with accum_out giving the
softmax denominator, reciprocal + per-row scale, PE transposes of the 3
window chunks into one bf16 PSUM tile, 3 AV matmuls against natural-layout
zero-padded V chunks accumulating both heads of a pair into one PSUM tile.

Depthwise conv K=31: 31 fused scalar_tensor_tensor taps (acc = shift*w +
acc) on DVE; an odd-shifted copy of the padded GLU output keeps every tap
4-byte aligned (bf16 2x DVE mode).
"""

import os
import sys
from contextlib import ExitStack

for _p in ("/opt/trn_rl_repo",):
    if _p not in sys.path:
        sys.path.insert(0, _p)

import numpy as np
import ml_dtypes

import concourse.bass as bass
import concourse.tile as tile
from concourse import bacc
from concourse import mybir
from concourse.bass_utils import run_bass_kernel_spmd

BF16 = mybir.dt.bfloat16
F32 = mybir.dt.float32
AF = mybir.ActivationFunctionType
OP = mybir.AluOpType

L, D, H, T, B = 4, 512, 8, 1024, 8
FF = 4 * D            # 2048
EC = 2 * D            # 1024 conv channels
KK = 31               # conv kernel size
WIN = 64              # attention window
DH = D // H           # 64
P = 128
DC = D // P           # 4 feature chunks
FC = FF // P          # 16
CC = EC // P          # 8
TB = T // P           # 8 token blocks
NT = 512              # matmul moving free dim
TC = T // NT          # 2 t-columns
KW = 3 * P            # 384: attention window width
EPS = 1e-5

TRACE = False          # set by test.py for profiling runs
TRACE_KW = {}
LAST_RESULT = None     # BassKernelResults of last run (read by test.py)
LAYERS = int(os.environ.get("CONF_LAYERS", str(L)))
PHASES = os.environ.get("CONF_PHASES", "fac2b")

# bias row indices in the packed bias tensor
BR_F1B1, BR_F1B2, BR_Q, BR_K, BR_V, BR_O, BR_P1, BR_P2, BR_F2B1, BR_F2B2 = range(10)


# transposed-scores attention tables: per key-block kb the valid query range
QL = [max(0, kb * P - WIN // 2) for kb in range(TB)]
QH = [min(T, kb * P + P + WIN // 2) for kb in range(TB)]
AT_PAIRS = ((0, 1), (2, 3), (4, 5), (6, 7))


def _pair_masks():
    """[128, w] multiplicative band masks for each kb-pair tile (bf16).

    Pair tile for (a, b): cols [0, wa) hold scoresT for kb=a over queries
    [QL[a], QH[a]); cols [wa, wa+wb) for kb=b.  Row i is key k = kb*128+i.
    """
    masks = []
    for a, b in AT_PAIRS:
        wa, wb = QH[a] - QL[a], QH[b] - QL[b]
        m = np.zeros((P, wa + wb), np.float32)
        for off, kb, w in ((0, a, wa), (wa, b, wb)):
            i = np.arange(P)[:, None]
            j = np.arange(w)[None, :]
            m[:, off : off + w] = np.abs((QL[kb] + j) - (kb * P + i)) <= WIN // 2
        masks.append(np.ascontiguousarray(m.astype(ml_dtypes.bfloat16)))
    return masks


def _av_pieces():
    """Per q-tile: list of (kb, lo, hi, start) AV matmul pieces.

    Fresh bands [QH[kb-1], QH[kb]) are each column's first writer
    (start=True); accum bands [QL[kb], QH[kb-1]) accumulate."""
    out = []
    for qt in range(TC):
        q0, q1 = qt * NT, (qt + 1) * NT
        pieces = []
        prev = 0
        for kb in range(TB):
            lo, hi = max(prev, q0), min(QH[kb], q1)
            if hi > lo:
                pieces.append((kb, lo, hi, True))
            if kb > 0:
                lo2, hi2 = max(QL[kb], q0), min(prev, q1)
                if hi2 > lo2:
                    pieces.append((kb, lo2, hi2, False))
            prev = QH[kb]
        out.append(pieces)
    return out


def build_program(flags):
    ln_gen = not flags["ln_trivial"]
    bias_gen = not flags["bias_trivial"]
    fin_gen = not flags["final_trivial"]
    dwb_gen = not flags["dwb_trivial"]

    nc = bacc.Bacc("TRN2", target_bir_lowering=False, debug=False)

    xt_d = nc.dram_tensor("x_t", [D, T], F32, kind="ExternalInput").ap()
    out_d = nc.dram_tensor("out_t", [D, T], F32, kind="ExternalOutput").ap()

    def win(name, shape):
        return nc.dram_tensor(name, shape, BF16, kind="ExternalInput").ap()

    w_f1a = win("f1w1", [L, D, FF])
    w_f1b = win("f1w2", [L, FF, D])
    w_f2a = win("f2w1", [L, D, FF])
    w_f2b = win("f2w2", [L, FF, D])
    w_q = win("wq", [L, D, D])
    w_kk = win("wk", [L, D, D])
    w_v = win("wv", [L, D, D])
    w_o = win("wo", [L, D, D])
    w_p1 = win("pw1", [L, D, 2 * EC])
    w_p2 = win("pw2", [L, EC, D])
    w_dw = nc.dram_tensor("dw", [L, P, CC, KK], F32, kind="ExternalInput").ap()
    w_gn = nc.dram_tensor("gn_aff", [L, 2, EC], F32, kind="ExternalInput").ap()
    w_gains = (nc.dram_tensor("ln_gains", [L, 10, D], F32, kind="ExternalInput").ap()
                if ln_gen else None)
    w_fin = (nc.dram_tensor("final_aff", [2, D], F32, kind="ExternalInput").ap()
             if fin_gen else None)
    w_bias = win("biases", [L, 10, 2 * EC]) if bias_gen else None
    w_dwb = (nc.dram_tensor("dwb", [L, P, CC], F32, kind="ExternalInput").ap()
             if dwb_gen else None)

    pm = _pair_masks()
    pmask_d = [nc.inline_tensor(pm[i], f"pmask{i}").ap() for i in range(4)]
    id_d = nc.inline_tensor(np.eye(P, dtype=ml_dtypes.bfloat16), "ident").ap()
    ones_d = nc.inline_tensor(np.ones((P, NT), dtype=ml_dtypes.bfloat16), "ones").ap()

    with tile.TileContext(nc) as tc, ExitStack() as ctx:
        pers = ctx.enter_context(tc.tile_pool(name="pers", bufs=1))
        wpool = ctx.enter_context(tc.tile_pool(name="w", bufs=1))
        spool = ctx.enter_context(tc.tile_pool(name="stat", bufs=1))
        hpool = ctx.enter_context(tc.tile_pool(name="h1", bufs=3))
        tpool = ctx.enter_context(tc.tile_pool(name="tanh", bufs=3))
        atpool = ctx.enter_context(tc.tile_pool(name="attn", bufs=2))
        smpool = ctx.enter_context(tc.tile_pool(name="small", bufs=8))
        cpool = ctx.enter_context(tc.tile_pool(name="conv", bufs=2))
        psum = ctx.enter_context(tc.tile_pool(name="ps", bufs=6, space="PSUM"))
        psc = ctx.enter_context(tc.tile_pool(name="psc", bufs=2, space="PSUM"))

        ident = pers.tile([P, P], BF16, tag="ident")
        nc.sync.dma_start(ident, id_d)
        ones = pers.tile([P, NT], BF16, tag="ones")
        nc.sync.dma_start(ones, ones_d)
        pmask = []
        for i in range(4):
            bt = pers.tile([P, pm[i].shape[1]], BF16, tag=f"pmask{i}")
            nc.sync.dma_start(bt, pmask_d[i])
            pmask.append(bt)

        x = pers.tile([P, DC, T], BF16, tag="x")
        with tc.tile_pool(name="xin", bufs=1) as xinp:
            for kc in range(DC):
                xf = xinp.tile([P, T], F32, name=f"xf{kc}", tag="xf")
                nc.sync.dma_start(
                    xf, xt_d.rearrange("(c p) t -> c p t", p=P)[kc])
                nc.vector.tensor_copy(out=x[:, kc], in_=xf)

        xh = pers.tile([P, DC, T], BF16, tag="xhat")
        qT = pers.tile([P, DC, T], BF16, tag="qT")
        kT = pers.tile([P, DC, T], BF16, tag="kT")
        # V in natural layout per key-block, 8 heads x 66 cols: col 64 = 1.0
        # (ones column makes the AV matmul also emit the softmax denominator)
        vno = pers.tile([P, TB, H, 66], BF16, tag="vno")
        oT = pers.tile([P, DC, T], BF16, tag="oT")
        c2 = pers.tile([P, CC, T], BF16, tag="c2")
        nc.vector.memset(vno[:, :, :, 64:65], 1.0)
        nc.vector.memset(vno[:, :, :, 65:66], 0.0)
        zrow = pers.tile([1, NT], BF16, tag="zrow")
        nc.vector.memset(zrow, 0.0)

        def ln_rstd(var, out_r, niter=2):
            nc.vector.tensor_scalar(
                out=out_r, in0=var, scalar1=-0.5, scalar2=1.5,
                op0=OP.mult, op1=OP.add)
            t1 = spool.tile(list(var.shape), F32, tag="nt1")
            for _ in range(niter):
                nc.vector.tensor_tensor(t1, out_r, out_r, OP.mult)
                nc.vector.scalar_tensor_tensor(
                    out=t1, in0=t1, scalar=-0.5, in1=var, op0=OP.mult, op1=OP.mult)
                nc.vector.scalar_tensor_tensor(
                    out=out_r, in0=t1, scalar=1.5, in1=out_r, op0=OP.add, op1=OP.mult)

        def emit_ln(src, dst, lidx, which, out_stream=None, fin_sb=None):
            """LN over the feature (partition) axis of src -> dst, both
            [P, DC, T] bf16.  which selects the gain row pair."""
            x2 = spool.tile([P, CC, T], BF16, tag="x2")
            for kc in range(DC):
                nc.vector.tensor_tensor(x2[:, kc], src[:, kc], src[:, kc], OP.mult)
            r_bf = spool.tile([P, T], BF16, tag="r_bf")
            mr_bf = spool.tile([P, T], BF16, tag="mr_bf")
            for tci in range(TC):
                sl = slice(tci * NT, (tci + 1) * NT)
                ps_s = psum.tile([P, NT], F32, tag="mm")
                ps_q = psum.tile([P, NT], F32, tag="mm")
                for kc in range(DC):
                    nc.tensor.matmul(ps_s, lhsT=ones[:, 0:P], rhs=src[:, kc, sl],
                                     start=(kc == 0), stop=(kc == DC - 1))
                for kc in range(DC):
                    nc.tensor.matmul(ps_q, lhsT=ones[:, 0:P], rhs=x2[:, kc, sl],
                                     start=(kc == 0), stop=(kc == DC - 1))
                s_sb = spool.tile([P, NT], F32, tag="s_sb")
                nc.vector.tensor_copy(out=s_sb, in_=ps_s)
                msq = spool.tile([P, NT], F32, tag="msq")
                nc.vector.scalar_tensor_tensor(
                    out=msq, in0=s_sb, scalar=1.0 / (D * D), in1=s_sb,
                    op0=OP.mult, op1=OP.mult)
                var = spool.tile([P, NT], F32, tag="var")
                nc.vector.scalar_tensor_tensor(
                    out=var, in0=ps_q, scalar=1.0 / D, in1=msq,
                    op0=OP.mult, op1=OP.subtract)
                r = spool.tile([P, NT], F32, tag="r")
                ln_rstd(var, r)
                nc.vector.tensor_copy(out=r_bf[:, sl], in_=r)
                mr = spool.tile([P, NT], F32, tag="mr")
                nc.vector.scalar_tensor_tensor(
                    out=mr, in0=s_sb, scalar=1.0 / D, in1=r, op0=OP.mult, op1=OP.mult)
                nc.vector.tensor_copy(out=mr_bf[:, sl], in_=mr)
            g_sb = None
            if w_gains is not None:
                g_sb = spool.tile([P, 2, DC], F32, tag="g_sb")
                nc.sync.dma_start(
                    g_sb, w_gains[lidx, 2 * which : 2 * which + 2]
                    .rearrange("g (c p) -> p g c", p=P))
            for kc in range(DC):
                for tci in range(TC):
                    sl = slice(tci * NT, (tci + 1) * NT)
                    u = tpool.tile([P, NT], BF16, tag="ln_u")
                    nc.vector.tensor_tensor(u, src[:, kc, sl], r_bf[:, sl], OP.mult)
                    tgt = dst[:, kc, sl]
                    nc.vector.tensor_tensor(tgt, u, mr_bf[:, sl], OP.subtract)
                    if g_sb is not None:
                        nc.scalar.activation(
                            out=tgt, in_=tgt, func=AF.Identity,
                            bias=g_sb[:, 1, kc : kc + 1], scale=g_sb[:, 0, kc : kc + 1])
                    if out_stream is not None:
                        pool_o, dview = out_stream
                        of = pool_o.tile([P, NT], F32, tag="of")
                        nc.vector.tensor_tensor(of, u, mr_bf[:, sl], OP.subtract)
                        if g_sb is not None:
                            nc.scalar.activation(
                                out=of, in_=of, func=AF.Identity,
                                bias=g_sb[:, 1, kc : kc + 1],
                                scale=g_sb[:, 0, kc : kc + 1])
                        if fin_sb is not None:
                            nc.scalar.activation(
                                out=of, in_=of, func=AF.Identity,
                                bias=fin_sb[:, 1, kc : kc + 1],
                                scale=fin_sb[:, 0, kc : kc + 1])
                        nc.sync.dma_start(dview[:, kc, sl], of)

        def load_w(dram, lidx, tag):
            _, fin, fout = dram.shape
            wt = wpool.tile([P, fin // P, fout], BF16, tag=tag)
            nc.sync.dma_start(wt, dram[lidx].rearrange("(c p) f -> p c f", p=P))
            return wt

        bias_sb = [None]

        def bias_mm(ps, row, mslice, tcslice_n):
            """Add bias row (features mslice) into psum via a K=1 matmul."""
            if bias_sb[0] is None:
                return
            nc.tensor.matmul(
                ps, lhsT=bias_sb[0][0:1, row, mslice], rhs=ones[0:1, 0:tcslice_n],
                start=False, stop=True, skip_group_check=True)

        def emit_ffn(wa_d, wb_d, rows, lidx, src):
            w1 = load_w(wa_d, lidx, "w1")
            w2 = load_w(wb_d, lidx, "w2")
            for tci in range(TC):
                sl = slice(tci * NT, (tci + 1) * NT)
                acc = [psum.tile([P, NT], F32, tag="mm", name=f"acc{i}")
                       for i in range(DC)]
                for m in range(FC):
                    ph = psum.tile([P, NT], F32, tag="mm")
                    for kc in range(DC):
                        nc.tensor.matmul(
                            ph, lhsT=w1[:, kc, m * P : (m + 1) * P], rhs=src[:, kc, sl],
                            start=(kc == 0), stop=(kc == DC - 1 and not bias_gen))
                    bias_mm(ph, rows[0], slice(m * P, (m + 1) * P), NT)
                    hb = hpool.tile([P, NT], BF16, tag="h1")
                    nc.scalar.activation(out=hb, in_=ph, func=AF.Silu)
                    for dcc in range(DC):
                        nc.tensor.matmul(
                            acc[dcc], lhsT=w2[:, m, dcc * P : (dcc + 1) * P], rhs=hb,
                            start=(m == 0), stop=(m == FC - 1 and not bias_gen),
                            skip_group_check=True)
                for dcc in range(DC):
                    bias_mm(acc[dcc], rows[1], slice(dcc * P, (dcc + 1) * P), NT)
                    nc.vector.scalar_tensor_tensor(
                        out=x[:, dcc, sl], in0=acc[dcc], scalar=1.0,
                        in1=x[:, dcc, sl], op0=OP.bypass, op1=OP.add)

        for l in range(LAYERS):
            if bias_gen:
                bt = wpool.tile([1, 10, 2 * EC], BF16, tag="bias")
                nc.sync.dma_start(bt, w_bias[l])
                bias_sb[0] = bt

            # ===== FFN1 (half residual) =====
            if "f" in PHASES:
                if l == 0 or ln_gen:
                    emit_ln(x, xh, l, 0)
                    src1 = xh
                else:
                    src1 = x  # already unit-normalized by previous blk LN
                emit_ffn(w_f1a, w_f1b, (BR_F1B1, BR_F1B2), l, src1)

            # ===== local windowed MHSA =====
            if "a" in PHASES or "A" in PHASES:
                emit_ln(x, xh, l, 1)
                wq = load_w(w_q, l, "wq")
                wk = load_w(w_kk, l, "wk")
                wv = load_w(w_v, l, "wv")
                wo = load_w(w_o, l, "wo")
                for m in range(DC):
                    for tci in range(TC):
                        sl = slice(tci * NT, (tci + 1) * NT)
                        pq = psum.tile([P, NT], F32, tag="mm")
                        for kc in range(DC):
                            nc.tensor.matmul(
                                pq, lhsT=wq[:, kc, m * P : (m + 1) * P],
                                rhs=xh[:, kc, sl],
                                start=(kc == 0), stop=(kc == DC - 1 and not bias_gen))
                        bias_mm(pq, BR_Q, slice(m * P, (m + 1) * P), NT)
                        nc.scalar.copy(out=qT[:, m, sl], in_=pq)
                        pk = psum.tile([P, NT], F32, tag="mm")
                        for kc in range(DC):
                            nc.tensor.matmul(
                                pk, lhsT=wk[:, kc, m * P : (m + 1) * P],
                                rhs=xh[:, kc, sl],
                                start=(kc == 0), stop=(kc == DC - 1 and not bias_gen))
                        bias_mm(pk, BR_K, slice(m * P, (m + 1) * P), NT)
                        nc.vector.tensor_copy(out=kT[:, m, sl], in_=pk)
                for tb in range(TB):
                    pv = psum.tile([P, NT], F32, tag="mm")
                    for kc in range(DC):
                        nc.tensor.matmul(
                            pv, lhsT=xh[:, kc, tb * P : (tb + 1) * P],
                            rhs=wv[:, kc, 0:D],
                            start=(kc == 0), stop=(kc == DC - 1 and not bias_gen))
                    if bias_gen:
                        nc.tensor.matmul(
                            pv, lhsT=ones[0:1, 0:P], rhs=bias_sb[0][0:1, BR_V, 0:D],
                            start=False, stop=True, skip_group_check=True)
                    nc.vector.tensor_copy(
                        out=vno[:, tb, :, 0:64],
                        in_=pv.rearrange("p (h d) -> p h d", d=DH))
                av_pieces = _av_pieces()
                for h in range(H):
                    hp, hh = divmod(h, 2)
                    pr = slice(hh * DH, (hh + 1) * DH)
                    at_tiles = []
                    for pi, (a, b) in enumerate(AT_PAIRS):
                        wa, wb = QH[a] - QL[a], QH[b] - QL[b]
                        sc = psc.tile([P, wa + wb], F32, tag="sc",
                                      name=f"sc{pi}")
                        nc.tensor.matmul(
                            sc[:, 0:wa], lhsT=kT[pr, hp, a * P : (a + 1) * P],
                            rhs=qT[pr, hp, QL[a] : QH[a]], start=True, stop=False)
                        nc.tensor.matmul(
                            sc[:, wa : wa + wb],
                            lhsT=kT[pr, hp, b * P : (b + 1) * P],
                            rhs=qT[pr, hp, QL[b] : QH[b]], start=False, stop=True,
                            skip_group_check=True)
                        at = atpool.tile([P, wa + wb], BF16, tag=f"at{pi}")
                        nc.scalar.activation(out=at, in_=sc, func=AF.Exp)
                        nc.vector.tensor_tensor(at, at, pmask[pi], OP.mult)
                        at_tiles.append(at)
                    for qt in range(TC):
                        av = psum.tile([P, NT], F32, tag="mm", name="av")
                        nc.tensor.matmul(
                            av[0:65, :], lhsT=ones[0:1, 0:65], rhs=zrow,
                            start=True, stop=False, skip_group_check=True)
                        npc = len(av_pieces[qt])
                        for i, (kb, lo, hi, st) in enumerate(av_pieces[qt]):
                            pi, sub = divmod(kb, 2)
                            a = AT_PAIRS[pi][0]
                            off = ((QH[a] - QL[a]) if sub else 0) + (lo - QL[kb])
                            nc.tensor.matmul(
                                av[0:65, lo - qt * NT : hi - qt * NT],
                                lhsT=vno[:, kb, h, 0:65],
                                rhs=at_tiles[pi][:, off : off + (hi - lo)],
                                start=False, stop=(i == npc - 1),
                                skip_group_check=True)
                        r_sb = atpool.tile([1, NT], F32, tag="r")
                        nc.vector.reciprocal(out=r_sb, in_=av[64:65, :])
                        rb_sb = atpool.tile([P, NT], F32, tag="rb")
                        nc.gpsimd.partition_broadcast(rb_sb, r_sb)
                        nc.vector.tensor_tensor(
                            oT[pr, hp, qt * NT : (qt + 1) * NT],
                            av[0:64, :], rb_sb[0:64, :], OP.mult)
                if "A" not in PHASES:
                    for tci in range(TC):
                        sl = slice(tci * NT, (tci + 1) * NT)
                        for m in range(DC):
                            pp = psum.tile([P, NT], F32, tag="mm")
                            for kc in range(DC):
                                nc.tensor.matmul(
                                    pp, lhsT=wo[:, kc, m * P : (m + 1) * P],
                                    rhs=oT[:, kc, sl],
                                    start=(kc == 0), stop=(kc == DC - 1 and not bias_gen))
                            bias_mm(pp, BR_O, slice(m * P, (m + 1) * P), NT)
                            nc.vector.scalar_tensor_tensor(
                                out=x[:, m, sl], in0=pp, scalar=1.0, in1=x[:, m, sl],
                                op0=OP.bypass, op1=OP.add)

            # ===== convolution module =====
            if "c" in PHASES:
                emit_ln(x, xh, l, 2)
                p1 = load_w(w_p1, l, "w1")
                p2 = load_w(w_p2, l, "w2")
                dwt = wpool.tile([P, CC, KK], F32, tag="dw")
                nc.sync.dma_start(dwt, w_dw[l])
                dwb_sb = None
                if dwb_gen:
                    dwb_sb = wpool.tile([P, CC], F32, tag="dwb")
                    nc.sync.dma_start(dwb_sb, w_dwb[l])
                for m in range(CC):
                    cp = cpool.tile([P, KK - 1 + T + 1], BF16, tag="cp")
                    co = cpool.tile([P, KK - 1 + T + 1], BF16, tag="co")
                    nc.vector.memset(cp[:, 0 : KK // 2], 0.0)
                    nc.vector.memset(cp[:, KK // 2 + T :], 0.0)
                    for tci in range(TC):
                        sl = slice(tci * NT, (tci + 1) * NT)
                        pb = psum.tile([P, NT], F32, tag="mm")
                        for kc in range(DC):
                            nc.tensor.matmul(
                                pb, lhsT=p1[:, kc, EC + m * P : EC + (m + 1) * P],
                                rhs=xh[:, kc, sl],
                                start=(kc == 0), stop=(kc == DC - 1 and not bias_gen))
                        bias_mm(pb, BR_P1, slice(EC + m * P, EC + (m + 1) * P), NT)
                        tb_ = tpool.tile([P, NT], BF16, tag="th")
                        nc.scalar.activation(out=tb_, in_=pb, func=AF.Tanh, scale=0.5)
                        pa = psum.tile([P, NT], F32, tag="mm")
                        for kc in range(DC):
                            nc.tensor.matmul(
                                pa, lhsT=p1[:, kc, m * P : (m + 1) * P],
                                rhs=xh[:, kc, sl],
                                start=(kc == 0), stop=(kc == DC - 1 and not bias_gen))
                        bias_mm(pa, BR_P1, slice(m * P, (m + 1) * P), NT)
                        nc.vector.scalar_tensor_tensor(
                            out=cp[:, KK // 2 + tci * NT : KK // 2 + (tci + 1) * NT],
                            in0=tb_, scalar=1.0, in1=pa, op0=OP.add, op1=OP.mult)
                    nc.vector.tensor_copy(out=co[:, 0 : KK - 1 + T],
                                          in_=cp[:, 1 : KK + T])
                    strip = wpool.tile([P, KK, P], BF16, tag="strip")
                    for kk in range(KK):
                        nc.vector.tensor_scalar_mul(
                            out=strip[:, kk, :], in0=ident,
                            scalar1=dwt[:, m, kk : kk + 1])
                    for tci in range(TC):
                        pc = psum.tile([P, NT], F32, tag="mm")
                        for kk in range(KK):
                            rhs = (cp[:, kk + tci * NT : kk + tci * NT + NT]
                                   if kk % 2 == 0 else
                                   co[:, kk - 1 + tci * NT : kk - 1 + tci * NT + NT])
                            nc.tensor.matmul(pc, lhsT=strip[:, kk, :], rhs=rhs,
                                             start=(kk == 0), stop=(kk == KK - 1),
                                             skip_group_check=True)
                        csl = c2[:, m, tci * NT : (tci + 1) * NT]
                        if dwb_sb is not None:
                            nc.vector.tensor_scalar_add(out=csl, in0=pc,
                                                        scalar1=dwb_sb[:, m : m + 1])
                        else:
                            nc.vector.tensor_copy(out=csl, in_=pc)
                # GroupNorm(1 group over [EC, T]) + silu fused
                cs = spool.tile([P, CC, T], BF16, tag="x2")
                for m in range(CC):
                    nc.vector.tensor_tensor(cs[:, m], c2[:, m], c2[:, m], OP.mult)
                parts = []
                for tci in range(TC):
                    sl = slice(tci * NT, (tci + 1) * NT)
                    ps_s = psum.tile([P, NT], F32, tag="mm")
                    ps_q = psum.tile([P, NT], F32, tag="mm")
                    for m in range(CC):
                        nc.tensor.matmul(ps_s, lhsT=ones[:, 0:P], rhs=c2[:, m, sl],
                                         start=(m == 0), stop=(m == CC - 1))
                    for m in range(CC):
                        nc.tensor.matmul(ps_q, lhsT=ones[:, 0:P], rhs=cs[:, m, sl],
                                         start=(m == 0), stop=(m == CC - 1))
                    rs = smpool.tile([P, 1], F32, tag=f"gs{tci}")
                    rq = smpool.tile([P, 1], F32, tag=f"gq{tci}")
                    nc.vector.tensor_reduce(out=rs, in_=ps_s,
                                            axis=mybir.AxisListType.X, op=OP.add)
                    nc.vector.tensor_reduce(out=rq, in_=ps_q,
                                            axis=mybir.AxisListType.X, op=OP.add)
                    parts.append((rs, rq))
                gs = smpool.tile([P, 1], F32, tag="gsum")
                gq = smpool.tile([P, 1], F32, tag="gqsum")
                nc.vector.tensor_tensor(gs, parts[0][0], parts[1][0], OP.add)
                nc.vector.tensor_tensor(gq, parts[0][1], parts[1][1], OP.add)
                mg = smpool.tile([P, 1], F32, tag="mg")
                nc.vector.tensor_scalar_mul(out=mg, in0=gs, scalar1=1.0 / (EC * T))
                msqg = smpool.tile([P, 1], F32, tag="msqg")
                nc.vector.tensor_tensor(msqg, mg, mg, OP.mult)
                varg = smpool.tile([P, 1], F32, tag="varg")
                nc.vector.scalar_tensor_tensor(
                    out=varg, in0=gq, scalar=1.0 / (EC * T), in1=msqg,
                    op0=OP.mult, op1=OP.subtract)
                nc.vector.tensor_scalar_add(out=varg, in0=varg, scalar1=EPS)
                rg = smpool.tile([P, 1], F32, tag="rg")
                ln_rstd(varg, rg, niter=14)
                # A = gn_g * r ; B = gn_b - m * A    (per-channel, [P, CC])
                gaff = spool.tile([P, 2, CC], F32, tag="gaff")
                nc.sync.dma_start(gaff, w_gn[l].rearrange("g (c p) -> p g c", p=P))
                a_t = spool.tile([P, CC], F32, tag="a_t")
                nc.vector.tensor_scalar_mul(out=a_t, in0=gaff[:, 0], scalar1=rg)
                mneg = smpool.tile([P, 1], F32, tag="mneg")
                nc.vector.tensor_scalar_mul(out=mneg, in0=mg, scalar1=-1.0)
                b_t = spool.tile([P, CC], F32, tag="b_t")
                nc.vector.scalar_tensor_tensor(
                    out=b_t, in0=a_t, scalar=mneg, in1=gaff[:, 1],
                    op0=OP.mult, op1=OP.add)
                b_bf = spool.tile([P, CC], BF16, tag="b_bf")
                nc.vector.tensor_copy(out=b_bf, in_=b_t)
                for m in range(CC):
                    y2 = cpool.tile([P, T], BF16, tag="y2")
                    nc.vector.scalar_tensor_tensor(
                        out=y2, in0=c2[:, m], scalar=a_t[:, m : m + 1],
                        in1=b_bf[:, m : m + 1].to_broadcast((P, T)),
                        op0=OP.mult, op1=OP.add)
                    nc.scalar.activation(out=c2[:, m], in_=y2, func=AF.Silu)
                for tci in range(TC):
                    sl = slice(tci * NT, (tci + 1) * NT)
                    for dcc in range(DC):
                        pp = psum.tile([P, NT], F32, tag="mm")
                        for m in range(CC):
                            nc.tensor.matmul(
                                pp, lhsT=p2[:, m, dcc * P : (dcc + 1) * P],
                                rhs=c2[:, m, sl],
                                start=(m == 0), stop=(m == CC - 1 and not bias_gen))
                        bias_mm(pp, BR_P2, slice(dcc * P, (dcc + 1) * P), NT)
                        nc.vector.scalar_tensor_tensor(
                            out=x[:, dcc, sl], in0=pp, scalar=1.0, in1=x[:, dcc, sl],
                            op0=OP.bypass, op1=OP.add)

            # ===== FFN2 (half residual) =====
            if "2" in PHASES:
                emit_ln(x, xh, l, 3)
                emit_ffn(w_f2a, w_f2b, (BR_F2B1, BR_F2B2), l, xh)

            # ===== per-block LN =====
            if "b" in PHASES:
                if l == LAYERS - 1:
                    with tc.tile_pool(name="outp", bufs=3) as op_:
                        fin_sb = None
                        if w_fin is not None:
                            fin_sb = spool.tile([P, 2, DC], F32, tag="fin_sb")
                            nc.sync.dma_start(
                                fin_sb, w_fin.rearrange("g (c p) -> p g c", p=P))
                        emit_ln(x, x, l, 4,
                                out_stream=(op_, out_d.rearrange(
                                    "(c p) t -> p c t", p=P)),
                                fin_sb=fin_sb)
                else:
                    emit_ln(x, x, l, 4)

        if "b" not in PHASES or LAYERS == 0:
            # debug path: dump current x (or oT for 'A') as output
            with tc.tile_pool(name="outp", bufs=3) as op_:
                srcd = oT if "A" in PHASES else x
                dview = out_d.rearrange("(c p) t -> p c t", p=P)
                for kc in range(DC):
                    for tci in range(TC):
                        sl = slice(tci * NT, (tci + 1) * NT)
                        of = op_.tile([P, NT], F32, tag="of")
                        nc.vector.tensor_copy(out=of, in_=srcd[:, kc, sl])
                        nc.sync.dma_start(dview[:, kc, sl], of)

    nc.finalize()
    return nc


_PROG_CACHE = {}


def _get_program(flags):
    key = tuple(sorted(flags.items())) + (LAYERS, PHASES)
    if key not in _PROG_CACHE:
        _PROG_CACHE[key] = build_program(flags)
    return _PROG_CACHE[key]


def kernel(**inputs):
    global LAST_RESULT
    f32 = lambda a: np.asarray(a, dtype=np.float32)
    bf = lambda a: np.ascontiguousarray(f32(a).astype(ml_dtypes.bfloat16))
    x = f32(inputs["x"])                       # [B, T, D]

    def triv(names_vals):
        return all(bool(np.all(f32(inputs[n]) == v)) for n, v in names_vals)

    ln_trivial = triv(
        [(f"{p}_ln_g", 1.0) for p in ("ffn1", "attn", "conv", "ffn2", "blk")]
        + [(f"{p}_ln_b", 0.0) for p in ("ffn1", "attn", "conv", "ffn2", "blk")])
    final_trivial = triv([("final_ln_g", 1.0), ("final_ln_b", 0.0)])
    bias_trivial = triv([(n, 0.0) for n in (
        "ffn1_b1", "ffn1_b2", "qkv_b", "outp_b", "pw1_b", "pw2_b",
        "ffn2_b1", "ffn2_b2")])
    dwb_trivial = triv([("dw_b", 0.0)])
    flags = dict(ln_trivial=ln_trivial, final_trivial=final_trivial,
                 bias_trivial=bias_trivial, dwb_trivial=dwb_trivial)

    nc = _get_program(flags)

    qkv = f32(inputs["qkv_w"])                # [L, D, 3D]
    dw = f32(inputs["dw_w"]).reshape(L, EC, KK) * 0.5
    dw = dw.reshape(L, CC, P, KK).transpose(0, 2, 1, 3)  # [L, P, CC, K]
    gn_aff = np.stack([f32(inputs["gn_g"]), f32(inputs["gn_b"])], axis=1)

    common = {
        "f1w1": bf(inputs["ffn1_w1"]),
        "f1w2": bf(f32(inputs["ffn1_w2"]) * 0.5),
        "f2w1": bf(inputs["ffn2_w1"]),
        "f2w2": bf(f32(inputs["ffn2_w2"]) * 0.5),
        "wq": bf(qkv[:, :, 0:D] * (DH ** -0.5)),
        "wk": bf(qkv[:, :, D : 2 * D]),
        "wv": bf(qkv[:, :, 2 * D : 3 * D]),
        "wo": bf(inputs["outp_w"]),
        "pw1": bf(inputs["pw1_w"]),
        "pw2": bf(inputs["pw2_w"]),
        "dw": np.ascontiguousarray(dw.astype(np.float32)),
        "gn_aff": np.ascontiguousarray(gn_aff.astype(np.float32)),
    }
    if not ln_trivial:
        rows = []
        for pfx in ("ffn1", "attn", "conv", "ffn2", "blk"):
            rows.append(f32(inputs[f"{pfx}_ln_g"]))
            rows.append(f32(inputs[f"{pfx}_ln_b"]))
        common["ln_gains"] = np.ascontiguousarray(
            np.stack(rows, axis=1).astype(np.float32))  # [L, 10, D]
    if not final_trivial:
        common["final_aff"] = np.ascontiguousarray(np.stack(
            [f32(inputs["final_ln_g"]), f32(inputs["final_ln_b"])]).astype(np.float32))
    if not bias_trivial:
        bias = np.zeros((L, 10, 2 * EC), np.float32)
        qb = f32(inputs["qkv_b"])
        bias[:, BR_F1B1, :FF] = f32(inputs["ffn1_b1"])
        bias[:, BR_F1B2, :D] = f32(inputs["ffn1_b2"]) * 0.5
        bias[:, BR_Q, :D] = qb[:, 0:D] * (DH ** -0.5)
        bias[:, BR_K, :D] = qb[:, D : 2 * D]
        bias[:, BR_V, :D] = qb[:, 2 * D : 3 * D]
        bias[:, BR_O, :D] = f32(inputs["outp_b"])
        bias[:, BR_P1, : 2 * EC] = f32(inputs["pw1_b"])
        bias[:, BR_P2, :D] = f32(inputs["pw2_b"])
        bias[:, BR_F2B1, :FF] = f32(inputs["ffn2_b1"])
        bias[:, BR_F2B2, :D] = f32(inputs["ffn2_b2"]) * 0.5
        common["biases"] = bf(bias)
    if not dwb_trivial:
        dwb = f32(inputs["dw_b"]).reshape(L, CC, P).transpose(0, 2, 1)
        common["dwb"] = np.ascontiguousarray(dwb.astype(np.float32))

    in_maps = []
    for c in range(B):
        m = dict(common)
        m["x_t"] = np.ascontiguousarray(x[c].T)   # [D, T] fp32
        in_maps.append(m)

    res = run_bass_kernel_spmd(
        nc, in_maps, core_ids=list(range(B)), trace=TRACE, **TRACE_KW)
    LAST_RESULT = res
    out = np.stack([r["out_t"].T for r in res.results]).astype(np.float32)
    return out


if __name__ == "__main__":
    rng = np.random.default_rng(0)
    ins = {"x": rng.standard_normal((B, T, D), dtype=np.float32)}
    # minimal smoke test requires full inputs; use test.py instead
    print("use test.py")



# revision 15
# speedup vs baseline: 1.0655x; 1.0655x over previous
"""ConformerDecoder Trainium2 Bass kernel.

Sharding: pure data-parallel over batch B=8 -> one sample per NeuronCore.

All activations live in "transposed" layout [feature-on-partitions, T-free]
so the entire matmul chain (ffn1 -> qkv -> attn -> out-proj -> conv pw1 ->
pw2 -> ffn2) runs with weights as natural lhsT operands and zero activation
transposes.  bf16 matmuls with fp32 PSUM accumulation.

Host-side one-time model formatting: cast weights to bf16, fold constant
scales (0.25 into ffn w2 = silu-half * residual-half; 0.5 into pw2 =
conv-silu half; 0.5 into dw = GLU-sigmoid half; 1/sqrt(DH) into Wq), build
band masks / identity / ones as inline const tensors.

LayerNorm (feature axis = partitions): per-token sums via ones[128,128]
matmuls on PE (output rows are 128-replicated broadcasts for free), rsqrt
via seeded Newton on DVE.  silu/sigmoid via Tanh so a single ACT table set
(exp_and_others: Exp+Tanh) serves the whole kernel -- no table switches.

Windowed attention (W=64): per (head, q-block of 128): one scores matmul
against a 3-chunk (384-wide) zero-padded K window, full-tile Exp on ACT,
multiplicative band-mask scalar_tensor_tensor with accum_out giving the
softmax denominator, reciprocal + per-row scale, PE transposes of the 3
window chunks into one bf16 PSUM tile, 3 AV matmuls against natural-layout
zero-padded V chunks accumulating both heads of a pair into one PSUM tile.

Depthwise conv K=31: 31 fused scalar_tensor_tensor taps (acc = shift*w +
acc) on DVE; an odd-shifted copy of the padded GLU output keeps every tap
4-byte aligned (bf16 2x DVE mode).
"""

import os
import sys
from contextlib import ExitStack

for _p in ("/opt/trn_rl_repo",):
    if _p not in sys.path:
        sys.path.insert(0, _p)

import numpy as np
import ml_dtypes

import concourse.bass as bass
import concourse.tile as tile
from concourse import bacc
from concourse import mybir
from concourse.bass_utils import run_bass_kernel_spmd

BF16 = mybir.dt.bfloat16
F32 = mybir.dt.float32
AF = mybir.ActivationFunctionType
OP = mybir.AluOpType

L, D, H, T, B = 4, 512, 8, 1024, 8
FF = 4 * D            # 2048
EC = 2 * D            # 1024 conv channels
KK = 31               # conv kernel size
WIN = 64              # attention window
DH = D // H           # 64
P = 128
DC = D // P           # 4 feature chunks
FC = FF // P          # 16
CC = EC // P          # 8
TB = T // P           # 8 token blocks
NT = 512              # matmul moving free dim
TC = T // NT          # 2 t-columns
KW = 3 * P            # 384: attention window width
EPS = 1e-5

TRACE = False          # set by test.py for profiling runs
TRACE_KW = {}
LAST_RESULT = None     # BassKernelResults of last run (read by test.py)
LAYERS = int(os.environ.get("CONF_LAYERS", str(L)))
PHASES = os.environ.get("CONF_PHASES", "fac2b")

# bias row indices in the packed bias tensor
BR_F1B1, BR_F1B2, BR_Q, BR_K, BR_V, BR_O, BR_P1, BR_P2, BR_F2B1, BR_F2B2 = range(10)


# transposed-scores attention tables: per key-block kb the valid query range
QL = [max(0, kb * P - WIN // 2) for kb in range(TB)]
QH = [min(T, kb * P + P + WIN // 2) for kb in range(TB)]
AT_PAIRS = ((0, 1), (2, 3), (4, 5), (6, 7))


def _pair_masks():
    """[128, w] multiplicative band masks for each kb-pair tile (bf16).

    Pair tile for (a, b): cols [0, wa) hold scoresT for kb=a over queries
    [QL[a], QH[a]); cols [wa, wa+wb) for kb=b.  Row i is key k = kb*128+i.
    """
    masks = []
    for a, b in AT_PAIRS:
        wa, wb = QH[a] - QL[a], QH[b] - QL[b]
        m = np.zeros((P, wa + wb), np.float32)
        for off, kb, w in ((0, a, wa), (wa, b, wb)):
            i = np.arange(P)[:, None]
            j = np.arange(w)[None, :]
            m[:, off : off + w] = np.abs((QL[kb] + j) - (kb * P + i)) <= WIN // 2
        masks.append(np.ascontiguousarray(m.astype(ml_dtypes.bfloat16)))
    return masks


def _av_pieces():
    """Per q-tile: list of (kb, lo, hi, start) AV matmul pieces.

    Fresh bands [QH[kb-1], QH[kb]) are each column's first writer
    (start=True); accum bands [QL[kb], QH[kb-1]) accumulate."""
    out = []
    for qt in range(TC):
        q0, q1 = qt * NT, (qt + 1) * NT
        pieces = []
        prev = 0
        for kb in range(TB):
            lo, hi = max(prev, q0), min(QH[kb], q1)
            if hi > lo:
                pieces.append((kb, lo, hi, True))
            if kb > 0:
                lo2, hi2 = max(QL[kb], q0), min(prev, q1)
                if hi2 > lo2:
                    pieces.append((kb, lo2, hi2, False))
            prev = QH[kb]
        out.append(pieces)
    return out


def build_program(flags):
    ln_gen = not flags["ln_trivial"]
    bias_gen = not flags["bias_trivial"]
    fin_gen = not flags["final_trivial"]
    dwb_gen = not flags["dwb_trivial"]

    nc = bacc.Bacc("TRN2", target_bir_lowering=False, debug=False)

    xt_d = nc.dram_tensor("x_t", [D, T], F32, kind="ExternalInput").ap()
    out_d = nc.dram_tensor("out_t", [D, T], F32, kind="ExternalOutput").ap()

    def win(name, shape):
        return nc.dram_tensor(name, shape, BF16, kind="ExternalInput").ap()

    w_f1a = win("f1w1", [L, D, FF])
    w_f1b = win("f1w2", [L, FF, D])
    w_f2a = win("f2w1", [L, D, FF])
    w_f2b = win("f2w2", [L, FF, D])
    w_q = win("wq", [L, D, D])
    w_kk = win("wk", [L, D, D])
    w_v = win("wv", [L, D, D])
    w_o = win("wo", [L, D, D])
    w_p1 = win("pw1", [L, D, 2 * EC])
    w_p2 = win("pw2", [L, EC, D])
    w_dw = nc.dram_tensor("dw", [L, P, CC, KK], F32, kind="ExternalInput").ap()
    w_gn = nc.dram_tensor("gn_aff", [L, 2, EC], F32, kind="ExternalInput").ap()
    w_gains = (nc.dram_tensor("ln_gains", [L, 10, D], F32, kind="ExternalInput").ap()
                if ln_gen else None)
    w_fin = (nc.dram_tensor("final_aff", [2, D], F32, kind="ExternalInput").ap()
             if fin_gen else None)
    w_bias = win("biases", [L, 10, 2 * EC]) if bias_gen else None
    w_dwb = (nc.dram_tensor("dwb", [L, P, CC], F32, kind="ExternalInput").ap()
             if dwb_gen else None)

    pm = _pair_masks()
    pmask_d = [nc.inline_tensor(pm[i], f"pmask{i}").ap() for i in range(4)]
    id_d = nc.inline_tensor(np.eye(P, dtype=ml_dtypes.bfloat16), "ident").ap()
    ones_d = nc.inline_tensor(np.ones((P, NT), dtype=ml_dtypes.bfloat16), "ones").ap()

    with tile.TileContext(nc) as tc, ExitStack() as ctx:
        pers = ctx.enter_context(tc.tile_pool(name="pers", bufs=1))
        wpool = ctx.enter_context(tc.tile_pool(name="w", bufs=1))
        spool = ctx.enter_context(tc.tile_pool(name="stat", bufs=1))
        hpool = ctx.enter_context(tc.tile_pool(name="h1", bufs=3))
        tpool = ctx.enter_context(tc.tile_pool(name="tanh", bufs=3))
        atpool = ctx.enter_context(tc.tile_pool(name="attn", bufs=2))
        smpool = ctx.enter_context(tc.tile_pool(name="small", bufs=8))
        cpool = ctx.enter_context(tc.tile_pool(name="conv", bufs=2))
        psum = ctx.enter_context(tc.tile_pool(name="ps", bufs=6, space="PSUM"))
        psc = ctx.enter_context(tc.tile_pool(name="psc", bufs=2, space="PSUM"))

        ident = pers.tile([P, P], BF16, tag="ident")
        nc.sync.dma_start(ident, id_d)
        ones = pers.tile([P, NT], BF16, tag="ones")
        nc.sync.dma_start(ones, ones_d)
        pmask = []
        for i in range(4):
            bt = pers.tile([P, pm[i].shape[1]], BF16, tag=f"pmask{i}")
            nc.sync.dma_start(bt, pmask_d[i])
            pmask.append(bt)

        x = pers.tile([P, DC, T], BF16, tag="x")
        with tc.tile_pool(name="xin", bufs=1) as xinp:
            for kc in range(DC):
                xf = xinp.tile([P, T], F32, name=f"xf{kc}", tag="xf")
                nc.sync.dma_start(
                    xf, xt_d.rearrange("(c p) t -> c p t", p=P)[kc])
                nc.vector.tensor_copy(out=x[:, kc], in_=xf)

        xh = pers.tile([P, DC, T], BF16, tag="xhat")
        qT = pers.tile([P, DC, T], BF16, tag="qT")
        kT = pers.tile([P, DC, T], BF16, tag="kT")
        # V in natural layout per key-block, 8 heads x 66 cols: col 64 = 1.0
        # (ones column makes the AV matmul also emit the softmax denominator)
        vno = pers.tile([P, TB, H, 66], BF16, tag="vno")
        oT = pers.tile([P, DC, T], BF16, tag="oT")
        c2 = pers.tile([P, CC, T], BF16, tag="c2")
        nc.vector.memset(vno[:, :, :, 64:65], 1.0)
        nc.vector.memset(vno[:, :, :, 65:66], 0.0)
        zrow = pers.tile([1, NT], BF16, tag="zrow")
        nc.vector.memset(zrow, 0.0)

        def ln_rstd(var, out_r, niter=2):
            nc.vector.tensor_scalar(
                out=out_r, in0=var, scalar1=-0.5, scalar2=1.5,
                op0=OP.mult, op1=OP.add)
            t1 = spool.tile(list(var.shape), F32, tag="nt1")
            for _ in range(niter):
                nc.vector.tensor_tensor(t1, out_r, out_r, OP.mult)
                nc.vector.scalar_tensor_tensor(
                    out=t1, in0=t1, scalar=-0.5, in1=var, op0=OP.mult, op1=OP.mult)
                nc.vector.scalar_tensor_tensor(
                    out=out_r, in0=t1, scalar=1.5, in1=out_r, op0=OP.add, op1=OP.mult)

        def emit_ln(src, dst, lidx, which, out_stream=None, fin_sb=None):
            """LN over the feature (partition) axis of src -> dst, both
            [P, DC, T] bf16.  which selects the gain row pair."""
            x2 = spool.tile([P, CC, T], BF16, tag="x2")
            for kc in range(DC):
                nc.vector.tensor_tensor(x2[:, kc], src[:, kc], src[:, kc], OP.mult)
            r_bf = spool.tile([P, T], BF16, tag="r_bf")
            mr_bf = spool.tile([P, T], BF16, tag="mr_bf")
            for tci in range(TC):
                sl = slice(tci * NT, (tci + 1) * NT)
                ps_s = psum.tile([P, NT], F32, tag="mm")
                ps_q = psum.tile([P, NT], F32, tag="mm")
                for kc in range(DC):
                    nc.tensor.matmul(ps_s, lhsT=ones[:, 0:P], rhs=src[:, kc, sl],
                                     start=(kc == 0), stop=(kc == DC - 1))
                for kc in range(DC):
                    nc.tensor.matmul(ps_q, lhsT=ones[:, 0:P], rhs=x2[:, kc, sl],
                                     start=(kc == 0), stop=(kc == DC - 1))
                s_sb = spool.tile([P, NT], F32, tag="s_sb")
                nc.vector.tensor_copy(out=s_sb, in_=ps_s)
                msq = spool.tile([P, NT], F32, tag="msq")
                nc.vector.scalar_tensor_tensor(
                    out=msq, in0=s_sb, scalar=1.0 / (D * D), in1=s_sb,
                    op0=OP.mult, op1=OP.mult)
                var = spool.tile([P, NT], F32, tag="var")
                nc.vector.scalar_tensor_tensor(
                    out=var, in0=ps_q, scalar=1.0 / D, in1=msq,
                    op0=OP.mult, op1=OP.subtract)
                r = spool.tile([P, NT], F32, tag="r")
                ln_rstd(var, r)
                nc.vector.tensor_copy(out=r_bf[:, sl], in_=r)
                mr = spool.tile([P, NT], F32, tag="mr")
                nc.vector.scalar_tensor_tensor(
                    out=mr, in0=s_sb, scalar=1.0 / D, in1=r, op0=OP.mult, op1=OP.mult)
                nc.vector.tensor_copy(out=mr_bf[:, sl], in_=mr)
            g_sb = None
            if w_gains is not None:
                g_sb = spool.tile([P, 2, DC], F32, tag="g_sb")
                nc.sync.dma_start(
                    g_sb, w_gains[lidx, 2 * which : 2 * which + 2]
                    .rearrange("g (c p) -> p g c", p=P))
            for kc in range(DC):
                for tci in range(TC):
                    sl = slice(tci * NT, (tci + 1) * NT)
                    u = tpool.tile([P, NT], BF16, tag="ln_u")
                    nc.vector.tensor_tensor(u, src[:, kc, sl], r_bf[:, sl], OP.mult)
                    tgt = dst[:, kc, sl]
                    nc.vector.tensor_tensor(tgt, u, mr_bf[:, sl], OP.subtract)
                    if g_sb is not None:
                        nc.scalar.activation(
                            out=tgt, in_=tgt, func=AF.Identity,
                            bias=g_sb[:, 1, kc : kc + 1], scale=g_sb[:, 0, kc : kc + 1])
                    if out_stream is not None:
                        pool_o, dview = out_stream
                        of = pool_o.tile([P, NT], F32, tag="of")
                        nc.vector.tensor_tensor(of, u, mr_bf[:, sl], OP.subtract)
                        if g_sb is not None:
                            nc.scalar.activation(
                                out=of, in_=of, func=AF.Identity,
                                bias=g_sb[:, 1, kc : kc + 1],
                                scale=g_sb[:, 0, kc : kc + 1])
                        if fin_sb is not None:
                            nc.scalar.activation(
                                out=of, in_=of, func=AF.Identity,
                                bias=fin_sb[:, 1, kc : kc + 1],
                                scale=fin_sb[:, 0, kc : kc + 1])
                        nc.sync.dma_start(dview[:, kc, sl], of)

        def load_w(dram, lidx, tag):
            _, fin, fout = dram.shape
            wt = wpool.tile([P, fin // P, fout], BF16, tag=tag)
            nc.sync.dma_start(wt, dram[lidx].rearrange("(c p) f -> p c f", p=P))
            return wt

        bias_sb = [None]

        def bias_mm(ps, row, mslice, tcslice_n):
            """Add bias row (features mslice) into psum via a K=1 matmul."""
            if bias_sb[0] is None:
                return
            nc.tensor.matmul(
                ps, lhsT=bias_sb[0][0:1, row, mslice], rhs=ones[0:1, 0:tcslice_n],
                start=False, stop=True, skip_group_check=True)

        def emit_ffn(wa_d, wb_d, rows, lidx, src):
            w1 = load_w(wa_d, lidx, "w1")
            w2 = load_w(wb_d, lidx, "w2")
            for tci in range(TC):
                sl = slice(tci * NT, (tci + 1) * NT)
                acc = [psum.tile([P, NT], F32, tag="mm", name=f"acc{i}")
                       for i in range(DC)]
                for m in range(FC):
                    ph = psum.tile([P, NT], F32, tag="mm")
                    for kc in range(DC):
                        nc.tensor.matmul(
                            ph, lhsT=w1[:, kc, m * P : (m + 1) * P], rhs=src[:, kc, sl],
                            start=(kc == 0), stop=(kc == DC - 1 and not bias_gen))
                    bias_mm(ph, rows[0], slice(m * P, (m + 1) * P), NT)
                    hb = hpool.tile([P, NT], BF16, tag="h1")
                    nc.scalar.activation(out=hb, in_=ph, func=AF.Silu)
                    for dcc in range(DC):
                        nc.tensor.matmul(
                            acc[dcc], lhsT=w2[:, m, dcc * P : (dcc + 1) * P], rhs=hb,
                            start=(m == 0), stop=(m == FC - 1 and not bias_gen),
                            skip_group_check=True)
                for dcc in range(DC):
                    bias_mm(acc[dcc], rows[1], slice(dcc * P, (dcc + 1) * P), NT)
                    nc.vector.scalar_tensor_tensor(
                        out=x[:, dcc, sl], in0=acc[dcc], scalar=1.0,
                        in1=x[:, dcc, sl], op0=OP.bypass, op1=OP.add)

        for l in range(LAYERS):
            if bias_gen:
                bt = wpool.tile([1, 10, 2 * EC], BF16, tag="bias")
                nc.sync.dma_start(bt, w_bias[l])
                bias_sb[0] = bt

            # ===== FFN1 (half residual) =====
            if "f" in PHASES:
                if l == 0 or ln_gen:
                    emit_ln(x, xh, l, 0)
                    src1 = xh
                else:
                    src1 = x  # already unit-normalized by previous blk LN
                emit_ffn(w_f1a, w_f1b, (BR_F1B1, BR_F1B2), l, src1)

            # ===== local windowed MHSA =====
            if "a" in PHASES or "A" in PHASES:
                emit_ln(x, xh, l, 1)
                wq = load_w(w_q, l, "wq")
                wk = load_w(w_kk, l, "wk")
                wv = load_w(w_v, l, "wv")
                wo = load_w(w_o, l, "wo")
                for m in range(DC):
                    for tci in range(TC):
                        sl = slice(tci * NT, (tci + 1) * NT)
                        pq = psum.tile([P, NT], F32, tag="mm")
                        for kc in range(DC):
                            nc.tensor.matmul(
                                pq, lhsT=wq[:, kc, m * P : (m + 1) * P],
                                rhs=xh[:, kc, sl],
                                start=(kc == 0), stop=(kc == DC - 1 and not bias_gen))
                        bias_mm(pq, BR_Q, slice(m * P, (m + 1) * P), NT)
                        nc.scalar.copy(out=qT[:, m, sl], in_=pq)
                        pk = psum.tile([P, NT], F32, tag="mm")
                        for kc in range(DC):
                            nc.tensor.matmul(
                                pk, lhsT=wk[:, kc, m * P : (m + 1) * P],
                                rhs=xh[:, kc, sl],
                                start=(kc == 0), stop=(kc == DC - 1 and not bias_gen))
                        bias_mm(pk, BR_K, slice(m * P, (m + 1) * P), NT)
                        nc.vector.tensor_copy(out=kT[:, m, sl], in_=pk)
                for tb in range(TB):
                    pv = psum.tile([P, NT], F32, tag="mm")
                    for kc in range(DC):
                        nc.tensor.matmul(
                            pv, lhsT=xh[:, kc, tb * P : (tb + 1) * P],
                            rhs=wv[:, kc, 0:D],
                            start=(kc == 0), stop=(kc == DC - 1 and not bias_gen))
                    if bias_gen:
                        nc.tensor.matmul(
                            pv, lhsT=ones[0:1, 0:P], rhs=bias_sb[0][0:1, BR_V, 0:D],
                            start=False, stop=True, skip_group_check=True)
                    nc.vector.tensor_copy(
                        out=vno[:, tb, :, 0:64],
                        in_=pv.rearrange("p (h d) -> p h d", d=DH))
                av_pieces = _av_pieces()
                for h in range(H):
                    hp, hh = divmod(h, 2)
                    pr = slice(hh * DH, (hh + 1) * DH)
                    at_tiles = []
                    for pi, (a, b) in enumerate(AT_PAIRS):
                        wa, wb = QH[a] - QL[a], QH[b] - QL[b]
                        sc = psc.tile([P, wa + wb], F32, tag="sc",
                                      name=f"sc{pi}")
                        nc.tensor.matmul(
                            sc[:, 0:wa], lhsT=kT[pr, hp, a * P : (a + 1) * P],
                            rhs=qT[pr, hp, QL[a] : QH[a]], start=True, stop=False)
                        nc.tensor.matmul(
                            sc[:, wa : wa + wb],
                            lhsT=kT[pr, hp, b * P : (b + 1) * P],
                            rhs=qT[pr, hp, QL[b] : QH[b]], start=False, stop=True,
                            skip_group_check=True)
                        at = atpool.tile([P, wa + wb], BF16, tag=f"at{pi}")
                        nc.scalar.activation(out=at, in_=sc, func=AF.Exp)
                        nc.vector.tensor_tensor(at, at, pmask[pi], OP.mult)
                        at_tiles.append(at)
                    for qt in range(TC):
                        av = psum.tile([P, NT], F32, tag="mm", name="av")
                        nc.tensor.matmul(
                            av[0:65, :], lhsT=ones[0:1, 0:65], rhs=zrow,
                            start=True, stop=False, skip_group_check=True)
                        npc = len(av_pieces[qt])
                        for i, (kb, lo, hi, st) in enumerate(av_pieces[qt]):
                            pi, sub = divmod(kb, 2)
                            a = AT_PAIRS[pi][0]
                            off = ((QH[a] - QL[a]) if sub else 0) + (lo - QL[kb])
                            nc.tensor.matmul(
                                av[0:65, lo - qt * NT : hi - qt * NT],
                                lhsT=vno[:, kb, h, 0:65],
                                rhs=at_tiles[pi][:, off : off + (hi - lo)],
                                start=False, stop=(i == npc - 1),
                                skip_group_check=True)
                        d_sb = atpool.tile([1, NT], F32, tag="r")
                        nc.vector.tensor_copy(out=d_sb, in_=av[64:65, :])
                        db = atpool.tile([P, NT], F32, tag="rb")
                        nc.gpsimd.partition_broadcast(db, d_sb)
                        rb_sb = atpool.tile([P, NT], F32, tag="rb2")
                        nc.vector.reciprocal_approx_fast(out=rb_sb, in_=db)
                        nc.vector.tensor_tensor(
                            oT[pr, hp, qt * NT : (qt + 1) * NT],
                            av[0:64, :], rb_sb[0:64, :], OP.mult)
                if "A" not in PHASES:
                    for tci in range(TC):
                        sl = slice(tci * NT, (tci + 1) * NT)
                        for m in range(DC):
                            pp = psum.tile([P, NT], F32, tag="mm")
                            for kc in range(DC):
                                nc.tensor.matmul(
                                    pp, lhsT=wo[:, kc, m * P : (m + 1) * P],
                                    rhs=oT[:, kc, sl],
                                    start=(kc == 0), stop=(kc == DC - 1 and not bias_gen))
                            bias_mm(pp, BR_O, slice(m * P, (m + 1) * P), NT)
                            nc.vector.scalar_tensor_tensor(
                                out=x[:, m, sl], in0=pp, scalar=1.0, in1=x[:, m, sl],
                                op0=OP.bypass, op1=OP.add)

            # ===== convolution module =====
            if "c" in PHASES:
                emit_ln(x, xh, l, 2)
                p1 = load_w(w_p1, l, "w1")
                p2 = load_w(w_p2, l, "w2")
                dwt = wpool.tile([P, CC, KK], F32, tag="dw")
                nc.sync.dma_start(dwt, w_dw[l])
                dwb_sb = None
                if dwb_gen:
                    dwb_sb = wpool.tile([P, CC], F32, tag="dwb")
                    nc.sync.dma_start(dwb_sb, w_dwb[l])
                for m in range(CC):
                    cp = cpool.tile([P, KK - 1 + T + 1], BF16, tag="cp")
                    co = cpool.tile([P, KK - 1 + T + 1], BF16, tag="co")
                    nc.vector.memset(cp[:, 0 : KK // 2], 0.0)
                    nc.vector.memset(cp[:, KK // 2 + T :], 0.0)
                    for tci in range(TC):
                        sl = slice(tci * NT, (tci + 1) * NT)
                        pb = psum.tile([P, NT], F32, tag="mm")
                        for kc in range(DC):
                            nc.tensor.matmul(
                                pb, lhsT=p1[:, kc, EC + m * P : EC + (m + 1) * P],
                                rhs=xh[:, kc, sl],
                                start=(kc == 0), stop=(kc == DC - 1 and not bias_gen))
                        bias_mm(pb, BR_P1, slice(EC + m * P, EC + (m + 1) * P), NT)
                        tb_ = tpool.tile([P, NT], BF16, tag="th")
                        nc.scalar.activation(out=tb_, in_=pb, func=AF.Tanh, scale=0.5)
                        pa = psum.tile([P, NT], F32, tag="mm")
                        for kc in range(DC):
                            nc.tensor.matmul(
                                pa, lhsT=p1[:, kc, m * P : (m + 1) * P],
                                rhs=xh[:, kc, sl],
                                start=(kc == 0), stop=(kc == DC - 1 and not bias_gen))
                        bias_mm(pa, BR_P1, slice(m * P, (m + 1) * P), NT)
                        nc.vector.scalar_tensor_tensor(
                            out=cp[:, KK // 2 + tci * NT : KK // 2 + (tci + 1) * NT],
                            in0=tb_, scalar=1.0, in1=pa, op0=OP.add, op1=OP.mult)
                    nc.vector.tensor_copy(out=co[:, 0 : KK - 1 + T],
                                          in_=cp[:, 1 : KK + T])
                    strip = wpool.tile([P, KK, P], BF16, tag="strip")
                    for kk in range(KK):
                        nc.vector.tensor_scalar_mul(
                            out=strip[:, kk, :], in0=ident,
                            scalar1=dwt[:, m, kk : kk + 1])
                    for tci in range(TC):
                        pc = psum.tile([P, NT], F32, tag="mm")
                        for kk in range(KK):
                            rhs = (cp[:, kk + tci * NT : kk + tci * NT + NT]
                                   if kk % 2 == 0 else
                                   co[:, kk - 1 + tci * NT : kk - 1 + tci * NT + NT])
                            nc.tensor.matmul(pc, lhsT=strip[:, kk, :], rhs=rhs,
                                             start=(kk == 0), stop=(kk == KK - 1),
                                             skip_group_check=True)
                        csl = c2[:, m, tci * NT : (tci + 1) * NT]
                        if dwb_sb is not None:
                            nc.vector.tensor_scalar_add(out=csl, in0=pc,
                                                        scalar1=dwb_sb[:, m : m + 1])
                        else:
                            nc.vector.tensor_copy(out=csl, in_=pc)
                # GroupNorm(1 group over [EC, T]) + silu fused
                cs = spool.tile([P, CC, T], BF16, tag="x2")
                for m in range(CC):
                    nc.vector.tensor_tensor(cs[:, m], c2[:, m], c2[:, m], OP.mult)
                parts = []
                for tci in range(TC):
                    sl = slice(tci * NT, (tci + 1) * NT)
                    ps_s = psum.tile([P, NT], F32, tag="mm")
                    ps_q = psum.tile([P, NT], F32, tag="mm")
                    for m in range(CC):
                        nc.tensor.matmul(ps_s, lhsT=ones[:, 0:P], rhs=c2[:, m, sl],
                                         start=(m == 0), stop=(m == CC - 1))
                    for m in range(CC):
                        nc.tensor.matmul(ps_q, lhsT=ones[:, 0:P], rhs=cs[:, m, sl],
                                         start=(m == 0), stop=(m == CC - 1))
                    rs = smpool.tile([P, 1], F32, tag=f"gs{tci}")
                    rq = smpool.tile([P, 1], F32, tag=f"gq{tci}")
                    nc.vector.tensor_reduce(out=rs, in_=ps_s,
                                            axis=mybir.AxisListType.X, op=OP.add)
                    nc.vector.tensor_reduce(out=rq, in_=ps_q,
                                            axis=mybir.AxisListType.X, op=OP.add)
                    parts.append((rs, rq))
                gs = smpool.tile([P, 1], F32, tag="gsum")
                gq = smpool.tile([P, 1], F32, tag="gqsum")
                nc.vector.tensor_tensor(gs, parts[0][0], parts[1][0], OP.add)
                nc.vector.tensor_tensor(gq, parts[0][1], parts[1][1], OP.add)
                mg = smpool.tile([P, 1], F32, tag="mg")
                nc.vector.tensor_scalar_mul(out=mg, in0=gs, scalar1=1.0 / (EC * T))
                msqg = smpool.tile([P, 1], F32, tag="msqg")
                nc.vector.tensor_tensor(msqg, mg, mg, OP.mult)
                varg = smpool.tile([P, 1], F32, tag="varg")
                nc.vector.scalar_tensor_tensor(
                    out=varg, in0=gq, scalar=1.0 / (EC * T), in1=msqg,
                    op0=OP.mult, op1=OP.subtract)
                nc.vector.tensor_scalar_add(out=varg, in0=varg, scalar1=EPS)
                rg = smpool.tile([P, 1], F32, tag="rg")
                ln_rstd(varg, rg, niter=14)
                # A = gn_g * r ; B = gn_b - m * A    (per-channel, [P, CC])
                gaff = spool.tile([P, 2, CC], F32, tag="gaff")
                nc.sync.dma_start(gaff, w_gn[l].rearrange("g (c p) -> p g c", p=P))
                a_t = spool.tile([P, CC], F32, tag="a_t")
                nc.vector.tensor_scalar_mul(out=a_t, in0=gaff[:, 0], scalar1=rg)
                mneg = smpool.tile([P, 1], F32, tag="mneg")
                nc.vector.tensor_scalar_mul(out=mneg, in0=mg, scalar1=-1.0)
                b_t = spool.tile([P, CC], F32, tag="b_t")
                nc.vector.scalar_tensor_tensor(
                    out=b_t, in0=a_t, scalar=mneg, in1=gaff[:, 1],
                    op0=OP.mult, op1=OP.add)
                for m in range(CC):
                    nc.scalar.activation(
                        out=c2[:, m], in_=c2[:, m], func=AF.Silu,
                        scale=a_t[:, m : m + 1], bias=b_t[:, m : m + 1])
                for tci in range(TC):
                    sl = slice(tci * NT, (tci + 1) * NT)
                    for dcc in range(DC):
                        pp = psum.tile([P, NT], F32, tag="mm")
                        for m in range(CC):
                            nc.tensor.matmul(
                                pp, lhsT=p2[:, m, dcc * P : (dcc + 1) * P],
                                rhs=c2[:, m, sl],
                                start=(m == 0), stop=(m == CC - 1 and not bias_gen))
                        bias_mm(pp, BR_P2, slice(dcc * P, (dcc + 1) * P), NT)
                        nc.vector.scalar_tensor_tensor(
                            out=x[:, dcc, sl], in0=pp, scalar=1.0, in1=x[:, dcc, sl],
                            op0=OP.bypass, op1=OP.add)

            # ===== FFN2 (half residual) =====
            if "2" in PHASES:
                emit_ln(x, xh, l, 3)
                emit_ffn(w_f2a, w_f2b, (BR_F2B1, BR_F2B2), l, xh)

            # ===== per-block LN =====
            if "b" in PHASES:
                if l == LAYERS - 1:
                    with tc.tile_pool(name="outp", bufs=3) as op_:
                        fin_sb = None
                        if w_fin is not None:
                            fin_sb = spool.tile([P, 2, DC], F32, tag="fin_sb")
                            nc.sync.dma_start(
                                fin_sb, w_fin.rearrange("g (c p) -> p g c", p=P))
                        emit_ln(x, x, l, 4,
                                out_stream=(op_, out_d.rearrange(
                                    "(c p) t -> p c t", p=P)),
                                fin_sb=fin_sb)
                else:
                    emit_ln(x, x, l, 4)

        if "b" not in PHASES or LAYERS == 0:
            # debug path: dump current x (or oT for 'A') as output
            with tc.tile_pool(name="outp", bufs=3) as op_:
                srcd = oT if "A" in PHASES else x
                dview = out_d.rearrange("(c p) t -> p c t", p=P)
                for kc in range(DC):
                    for tci in range(TC):
                        sl = slice(tci * NT, (tci + 1) * NT)
                        of = op_.tile([P, NT], F32, tag="of")
                        nc.vector.tensor_copy(out=of, in_=srcd[:, kc, sl])
                        nc.sync.dma_start(dview[:, kc, sl], of)

    nc.finalize()
    return nc


_PROG_CACHE = {}


def _get_program(flags):
    key = tuple(sorted(flags.items())) + (LAYERS, PHASES)
    if key not in _PROG_CACHE:
        _PROG_CACHE[key] = build_program(flags)
    return _PROG_CACHE[key]


def kernel(**inputs):
    global LAST_RESULT
    f32 = lambda a: np.asarray(a, dtype=np.float32)
    bf = lambda a: np.ascontiguousarray(f32(a).astype(ml_dtypes.bfloat16))
    x = f32(inputs["x"])                       # [B, T, D]

    def triv(names_vals):
        return all(bool(np.all(f32(inputs[n]) == v)) for n, v in names_vals)

    ln_trivial = triv(
        [(f"{p}_ln_g", 1.0) for p in ("ffn1", "attn", "conv", "ffn2", "blk")]
        + [(f"{p}_ln_b", 0.0) for p in ("ffn1", "attn", "conv", "ffn2", "blk")])
    final_trivial = triv([("final_ln_g", 1.0), ("final_ln_b", 0.0)])
    bias_trivial = triv([(n, 0.0) for n in (
        "ffn1_b1", "ffn1_b2", "qkv_b", "outp_b", "pw1_b", "pw2_b",
        "ffn2_b1", "ffn2_b2")])
    dwb_trivial = triv([("dw_b", 0.0)])
    flags = dict(ln_trivial=ln_trivial, final_trivial=final_trivial,
                 bias_trivial=bias_trivial, dwb_trivial=dwb_trivial)

    nc = _get_program(flags)

    qkv = f32(inputs["qkv_w"])                # [L, D, 3D]
    dw = f32(inputs["dw_w"]).reshape(L, EC, KK) * 0.5
    dw = dw.reshape(L, CC, P, KK).transpose(0, 2, 1, 3)  # [L, P, CC, K]
    gn_aff = np.stack([f32(inputs["gn_g"]), f32(inputs["gn_b"])], axis=1)

    common = {
        "f1w1": bf(inputs["ffn1_w1"]),
        "f1w2": bf(f32(inputs["ffn1_w2"]) * 0.5),
        "f2w1": bf(inputs["ffn2_w1"]),
        "f2w2": bf(f32(inputs["ffn2_w2"]) * 0.5),
        "wq": bf(qkv[:, :, 0:D] * (DH ** -0.5)),
        "wk": bf(qkv[:, :, D : 2 * D]),
        "wv": bf(qkv[:, :, 2 * D : 3 * D]),
        "wo": bf(inputs["outp_w"]),
        "pw1": bf(inputs["pw1_w"]),
        "pw2": bf(inputs["pw2_w"]),
        "dw": np.ascontiguousarray(dw.astype(np.float32)),
        "gn_aff": np.ascontiguousarray(gn_aff.astype(np.float32)),
    }
    if not ln_trivial:
        rows = []
        for pfx in ("ffn1", "attn", "conv", "ffn2", "blk"):
            rows.append(f32(inputs[f"{pfx}_ln_g"]))
            rows.append(f32(inputs[f"{pfx}_ln_b"]))
        common["ln_gains"] = np.ascontiguousarray(
            np.stack(rows, axis=1).astype(np.float32))  # [L, 10, D]
    if not final_trivial:
        common["final_aff"] = np.ascontiguousarray(np.stack(
            [f32(inputs["final_ln_g"]), f32(inputs["final_ln_b"])]).astype(np.float32))
    if not bias_trivial:
        bias = np.zeros((L, 10, 2 * EC), np.float32)
        qb = f32(inputs["qkv_b"])
        bias[:, BR_F1B1, :FF] = f32(inputs["ffn1_b1"])
        bias[:, BR_F1B2, :D] = f32(inputs["ffn1_b2"]) * 0.5
        bias[:, BR_Q, :D] = qb[:, 0:D] * (DH ** -0.5)
        bias[:, BR_K, :D] = qb[:, D : 2 * D]
        bias[:, BR_V, :D] = qb[:, 2 * D : 3 * D]
        bias[:, BR_O, :D] = f32(inputs["outp_b"])
        bias[:, BR_P1, : 2 * EC] = f32(inputs["pw1_b"])
        bias[:, BR_P2, :D] = f32(inputs["pw2_b"])
        bias[:, BR_F2B1, :FF] = f32(inputs["ffn2_b1"])
        bias[:, BR_F2B2, :D] = f32(inputs["ffn2_b2"]) * 0.5
        common["biases"] = bf(bias)
    if not dwb_trivial:
        dwb = f32(inputs["dw_b"]).reshape(L, CC, P).transpose(0, 2, 1)
        common["dwb"] = np.ascontiguousarray(dwb.astype(np.float32))

    in_maps = []
    for c in range(B):
        m = dict(common)
        m["x_t"] = np.ascontiguousarray(x[c].T)   # [D, T] fp32
        in_maps.append(m)

    res = run_bass_kernel_spmd(
        nc, in_maps, core_ids=list(range(B)), trace=TRACE, **TRACE_KW)
    LAST_RESULT = res
    out = np.stack([r["out_t"].T for r in res.results]).astype(np.float32)
    return out


if __name__ == "__main__":
    rng = np.random.default_rng(0)
    ins = {"x": rng.standard_normal((B, T, D), dtype=np.float32)}
    # minimal smoke test requires full inputs; use test.py instead
    print("use test.py")

